# revision 27
# baseline (speedup 1.0000x reference)
"""Multi-head self-attention (B=2, T=4096, D=768, H=12) on 8 TRN2 NeuronCores.

Sharding: (batch, head)-parallel. Core c (0..7) handles batch b=c//4 and the
3 heads h0=(c%4)*3 .. h0+2.  Each core computes Q/K/V projections for its
heads, full softmax(QK^T/sqrt(d))V attention, and a partial output projection
through its 192 rows of Wo.  The host sums the 4 partials per batch
(row-sharded Wo all-reduce done host-side); bo rides an augmented ones-row
on core c%4==0 only.

Per-core device pipeline:
  x.T [768,4096] -> Qt/Kt in [d,t] layout (heads pair-packed into 128
  partitions), V in natural [t,d] layout augmented with a leading ones
  column.  Scores are computed transposed, S^T[k,q], with two concurrent
  row-tiled matmuls (K=64 each); ACT applies exp(scale*s) PSUM->SBUF bf16;
  PV accumulates O'[65,512] = sum_k V'[k]^T P[k] where row 0 is the softmax
  denominator; a K=1 broadcast matmul + approx reciprocal + multiply
  normalizes; the Wo projection consumes the normalized O^T tiles directly.
"""

import os
import numpy as np
import ml_dtypes

B, T, D = 2, 4096, 768
H, DH = 12, 64
NCORES = 8
HPC = 3            # heads per core
KC = D // 128      # 6 contraction chunks for projections
NT = T // 512      # 8 q/n tiles of 512
TT = T // 128      # 32 k/t tiles of 128
EXPG = 2           # k-tiles per exp instruction (PSUM banks per S tile)

BF16 = ml_dtypes.bfloat16

_CACHE = {}

# Custom-DVE exp approximation on pre-scaled scores v = ALPHA * u:
#   out = (((v + C0)*v + C1)*v + C2)^4  ~=  K4 * exp(u/8),  |u/8| <= ~2.6
# The monic cubic comes from folding the leading minimax coefficient into
# the Q projection weights (ALPHA), so the op needs only 3 scalars and a
# single tensor stream.  The tiny global factor K4 cancels in softmax;
# the ACT path matches it via its free bias so both engines agree.
EXP_ALPHA = 0.017071602825145405
EXPC0 = 1.7311600515579881
EXPC1 = 1.832589311937295
EXPC2 = 0.99834169
EXP_LNK4 = -0.0035190274290640955
ACT_SCALE = (64 ** -0.5) / EXP_ALPHA  # exp(u/8) with u = v/ALPHA
_EXP_SHA_V3 = "4700acb2322239e1"


def _register_exp_op():
    from concourse import dve_ops
    from concourse.dve_spec import C0, C1, C2, Spec, Src0, sq

    for o in dve_ops.OPS:
        if o.name == "EXP_MONIC4_ANT":
            return o

    p = ((Src0 + C0) * Src0 + C1) * Src0 + C2
    body = sq(sq(p))

    def ref(in0, in1, s0, s1, imm2):
        x = in0.astype(np.float32)
        pv = ((x + s0) * x + s1) * x + imm2
        return ((pv * pv) ** 2).astype(np.float32)

    op = dve_ops.DveOp(
        "EXP_MONIC4_ANT",
        Spec(body=body, reference=ref),
        subdim=False,
        uops_sha={"v3": _EXP_SHA_V3},
    )
    dve_ops.OPS.append(op)
    dve_ops._SUB_OPCODE_FOR_NAME[op.name] = (
        dve_ops._CUSTOM_DVE_ROW_BASE + len(dve_ops.OPS) - 1
    )
    dve_ops.CUSTOM_DVE_SPECS[op.name] = op.spec
    return op


def _trace(nc, tc, mybir, tens, iters=1):
    skip = os.environ.get("MHSA_SKIP", "")
    import concourse.bass as bass
    from contextlib import ExitStack

    f32 = mybir.dt.float32
    bf16 = mybir.dt.bfloat16
    Exp = mybir.ActivationFunctionType.Exp
    PSUM = bass.MemorySpace.PSUM
    NG = TT // EXPG  # exp groups per (head, qtile) in the paired path
    exp_op = _register_exp_op()

    with ExitStack() as ctx:
        persist = ctx.enter_context(tc.tile_pool(name="persist", bufs=1))

        # ---- persistent SBUF ----
        x_ch = [
            persist.tile([128, T], bf16, name=f"xc{kc}") for kc in range(KC)
        ]
        w_q = persist.tile([128, KC, HPC * DH], bf16)
        w_k = persist.tile([128, KC, HPC * DH], bf16)
        w_v = persist.tile([128, KC, HPC * DH], bf16)
        bq01 = persist.tile([128, 1], f32)
        bq2 = persist.tile([64, 1], f32)
        bk01 = persist.tile([128, 1], f32)
        bk2v = persist.tile([128, 1], f32)  # h2 K bias on partitions 64:128
        bv_sb = persist.tile([1, HPC * DH], bf16)
        ones1 = persist.tile([1, 128], bf16)     # K=1 lhsT for V bias MM
        f32r = mybir.dt.float32r
        ones65 = persist.tile([DH + 1, DH + 1], f32r)  # row 64: K=1 denom bcast lhsT
        q01 = persist.tile([128, T], bf16)       # h0 rows 0:64, h1 rows 64:128
        k01 = persist.tile([128, T], bf16)
        q2 = persist.tile([128, T], bf16)        # h2, duplicated to rows 64:128
        k2 = persist.tile([128, T], bf16)
        v_sb = persist.tile([128, TT, HPC, 68], bf16)  # [1|V] per head
        # normalized O^T: on01 rows 0:64 = h0, 64:128 = h1; on2 rows 0:64 = h2,
        # row 64 = ones (rides bo through the phase-3 K=65 matmul)
        on01 = persist.tile([128, T], bf16)
        on2 = persist.tile([DH + 1, T], bf16)
        wo01_sb = persist.tile([128, D], bf16)
        wo2_sb = persist.tile([DH + 1, D], bf16)

        lnk4 = persist.tile([128, 1], f32)  # ACT exp bias matching the poly

        nc.vector.memset(v_sb[:, :, :, 64:65], 1.0)
        nc.vector.memset(lnk4[:], EXP_LNK4)

        dve_share = float(os.environ.get("MHSA_DVE_SHARE", "0.5"))
        exp_acc = [0.0]

        def do_exp(dst, src):
            exp_acc[0] += dve_share
            if exp_acc[0] >= 1.0 - 1e-9:
                exp_acc[0] -= 1.0
                nc.vector._custom_dve(
                    exp_op, out=dst, in0=src,
                    s0=EXPC0, s1=EXPC1, imm2=EXPC2,
                )
            else:
                nc.scalar.activation(
                    dst, src, Exp, scale=ACT_SCALE, bias=lnk4[:],
                )

        # ---- input DMAs ----
        xT, wqT, wkT, wvT, bq, bk, bv, wo01, wo2, onesb, ones65d, y = tens
        nc.sync.dma_start(ones1[:], onesb[0:1, 0:128])
        nc.sync.dma_start(on2[DH : DH + 1, :], onesb[:])
        nc.sync.dma_start(ones65[DH : DH + 1, :], ones65d[:])
        for kc in range(KC):
            r = slice(kc * 128, (kc + 1) * 128)
            nc.sync.dma_start(x_ch[kc][:], xT[r, :])
            nc.sync.dma_start(w_q[:, kc, :], wqT[r, :])
            nc.sync.dma_start(w_k[:, kc, :], wkT[r, :])
            nc.sync.dma_start(w_v[:, kc, :], wvT[r, :])
        nc.sync.dma_start(bq01[:], bq[0:128, :])
        nc.sync.dma_start(bq2[:], bq[128:192, :])
        nc.sync.dma_start(bk01[:], bk[0:128, :])
        nc.sync.dma_start(bk2v[64:128, :], bk[128:192, :])
        nc.sync.dma_start(bv_sb[:], bv[:])
        nc.sync.dma_start(wo01_sb[:], wo01[:])
        nc.sync.dma_start(wo2_sb[:], wo2[:])

        loop_cm = tc.For_i(0, iters, 1) if iters > 1 else None
        from contextlib import nullcontext
        with (loop_cm if loop_cm is not None else nullcontext()):
            # ---- Phase 1a: Q/K projections into [d, t] layout ----
            # pqb/pkb (64 rows each) run col-tiled concurrently in one pass
            with tc.tile_pool(name="pj", bufs=2, space=PSUM) as pj:
                for nt in range(NT):
                    s = slice(nt * 512, (nt + 1) * 512)
                    pqa = pj.tile([128, 512], f32, tag="pqa")
                    pka = pj.tile([128, 512], f32, tag="pka")
                    # h2 halves in separate banks but distinct col groups so
                    # the two M=64 matmul streams run concurrently on the PE
                    pq2 = pj.tile([128, 512], f32, tag="pq2")
                    pk2 = pj.tile([128, 512], f32, tag="pk2")
                    for kc in range(KC):
                        st, sp = kc == 0, kc == KC - 1
                        rhs = x_ch[kc][:, s]
                        nc.tensor.matmul(pqa[:], w_q[:, kc, 0:128], rhs, start=st, stop=sp)
                        nc.tensor.matmul(pka[:], w_k[:, kc, 0:128], rhs, start=st, stop=sp)
                        nc.tensor.matmul(pq2[0:64, :], w_q[:, kc, 128:192], rhs,
                                         start=st, stop=sp, tile_position=(0, 0))
                        nc.tensor.matmul(pk2[64:128, :], w_k[:, kc, 128:192], rhs,
                                         start=st, stop=sp, tile_position=(0, 64))
                    nc.vector.tensor_scalar_add(q01[:, s], pqa[:], bq01[:])
                    nc.vector.tensor_scalar_add(k01[:, s], pka[:], bk01[:])
                    nc.vector.tensor_scalar_add(q2[0:64, s], pq2[0:64, :], bq2[:])
                    nc.vector.tensor_scalar_add(k2[64:128, s], pk2[64:128, :],
                                                bk2v[64:128, :])

            # ---- Phase 1b: V projection into natural [t, d] layout ----
            with tc.tile_pool(name="pv", bufs=4, space=PSUM) as pvp:
                for tt in range(TT):
                    ts_ = slice(tt * 128, (tt + 1) * 128)
                    pvt = pvp.tile([128, HPC * DH], f32, tag="pvt")
                    nc.tensor.matmul(pvt[:], ones1[:], bv_sb[:], start=True, stop=False)
                    for kc in range(KC):
                        nc.tensor.matmul(
                            pvt[:], x_ch[kc][:, ts_], w_v[:, kc, :],
                            start=False, stop=kc == KC - 1,
                        )
                    nc.vector.tensor_copy(
                        v_sb[:, tt, :, 0:64],
                        pvt[:].rearrange("p (h d) -> p h d", h=HPC),
                    )

            # duplicate h2's Q/K to the other partition half for self-pairing
            nc.sync.dma_start(q2[64:128, :], q2[0:64, :])
            nc.sync.dma_start(k2[0:64, :], k2[64:128, :])

            # ---- Phase 2: attention per q-tile (exp split ACT/DVE) ----
            # DVE does ONLY exp + the short normalize tail here; everything
            # heavy (output projection, PSUM evacuation) moves to phase 3 so
            # the strict-FIFO DVE queue never blocks the next q-tile's exps.
            with (
                tc.tile_pool(name="spool", bufs=1, space=PSUM) as spool,
                tc.tile_pool(name="opool", bufs=1, space=PSUM) as opool,
                tc.tile_pool(name="mpool", bufs=1, space=PSUM) as mpool,
                tc.tile_pool(name="ppool", bufs=3) as ppool,
                tc.tile_pool(name="npool", bufs=2) as npool,
            ):
                for qt in range(NT):
                    qs = slice(qt * 512, (qt + 1) * 512)

                    def s_group(g, emit_pair):
                        """Emit EXPG steps of (2 score MMs) -> 2 exps -> caller PV."""
                        st = [
                            spool.tile([128, EXPG * 512], f32, tag=f"s{i}", name=f"s{i}")
                            for i in (0, 1)
                        ]
                        for j in range(EXPG):
                            emit_pair(st, g * EXPG + j, j)
                        pt = [
                            ppool.tile([128, EXPG * 512], bf16, tag=f"p{i}", name=f"p{i}")
                            for i in (0, 1)
                        ]
                        if "exp" not in skip:
                            do_exp(pt[0][:], st[0][:])
                            do_exp(pt[1][:], st[1][:])
                        return pt

                    # --- h0/h1 concurrently (row strips 0:64 / 64:128) ---
                    ol = [opool.tile([DH + 1, 512], f32, tag=f"o{i}", name=f"ol{i}") for i in (0, 1)]

                    def pair01(st, kt, j):
                        ks = slice(kt * 128, (kt + 1) * 128)
                        js = slice(j * 512, (j + 1) * 512)
                        nc.tensor.matmul(st[0][:, js], k01[0:64, ks], q01[0:64, qs],
                                         start=True, stop=True)
                        nc.tensor.matmul(st[1][:, js], k01[64:128, ks], q01[64:128, qs],
                                         start=True, stop=True)

                    for g in range(NG):
                        pt = s_group(g, pair01)
                        for j in range(EXPG if "pv" not in skip else 0):
                            kt = g * EXPG + j
                            js = slice(j * 512, (j + 1) * 512)
                            first = kt == 0
                            last = kt == TT - 1
                            nc.tensor.matmul(ol[0][:], v_sb[:, kt, 0, 0:65], pt[0][:, js],
                                             start=first, stop=last, skip_group_check=True)
                            nc.tensor.matmul(ol[1][:], v_sb[:, kt, 1, 0:65], pt[1][:, js],
                                             start=first, stop=last, skip_group_check=True)

                    norm_jobs = [(ol[0], on01[0:DH, :]), (ol[1], on01[DH : 2 * DH, :])]

                    # --- h2: even k-tiles on rows 0:64, odd on rows 64:128;
                    # both halves accumulate into ONE psum tile ---
                    o2 = opool.tile([DH + 1, 512], f32, tag="o2", name="o2")

                    def pair2(st, p, j):
                        ka = slice(2 * p * 128, (2 * p + 1) * 128)
                        kb = slice((2 * p + 1) * 128, (2 * p + 2) * 128)
                        js = slice(j * 512, (j + 1) * 512)
                        nc.tensor.matmul(st[0][:, js], k2[0:64, ka], q2[0:64, qs],
                                         start=True, stop=True)
                        nc.tensor.matmul(st[1][:, js], k2[64:128, kb], q2[64:128, qs],
                                         start=True, stop=True)

                    for g in range(NG // 2):
                        pt = s_group(g, pair2)
                        for j in range(EXPG if "pv" not in skip else 0):
                            p = g * EXPG + j
                            js = slice(j * 512, (j + 1) * 512)
                            nc.tensor.matmul(o2[:], v_sb[:, 2 * p, 2, 0:65], pt[0][:, js],
                                             start=p == 0, stop=False,
                                             skip_group_check=True)
                            nc.tensor.matmul(o2[:], v_sb[:, 2 * p + 1, 2, 0:65],
                                             pt[1][:, js],
                                             start=False, stop=p == TT // 2 - 1,
                                             skip_group_check=True)
                    if "pv" not in skip:
                        norm_jobs.append((o2, on2[0:DH, :]))
                    else:
                        norm_jobs = []

                    # --- normalize: broadcast denom row via K=1 matmul,
                    # reciprocal into SBUF, then mul (psum x sbuf) per head ---
                    for o_acc, on in norm_jobs:
                        lrow = npool.tile([DH + 1, 512], f32r, tag="lr")
                        nc.vector.tensor_copy(lrow[DH : DH + 1, :], o_acc[DH : DH + 1, :])
                        bc = mpool.tile([DH, 512], f32, tag="bc")
                        nc.tensor.matmul(
                            bc[:],
                            ones65[DH : DH + 1, 0:DH],
                            lrow[DH : DH + 1, :],
                            start=True, stop=True,
                        )
                        rc = npool.tile([DH, 512], f32, tag="rc")
                        nc.vector.reciprocal_approx_fast(rc[:], bc[:])
                        nc.vector.tensor_mul(on[:, qs], o_acc[0:DH, :], rc[:])

            # ---- Phase 3: output projection, y^T = Wo'^T @ O^T ----
            # K=128 (h0|h1) + K=65 (h2 + bias row) accumulating matmuls per
            # 128-wide d-chunk; PSUM evacuation split across ACT and DVE.
            if "wo" not in skip:
                with (
                    tc.tile_pool(name="ypsum", bufs=2, space=PSUM) as ypp,
                    tc.tile_pool(name="ysbp", bufs=3) as ysp,
                ):
                    for qt in range(NT):
                        qs = slice(qt * 512, (qt + 1) * 512)
                        for dc in range(D // 128):
                            ds = slice(dc * 128, (dc + 1) * 128)
                            yt = ypp.tile([128, 512], f32, tag="yt")
                            nc.tensor.matmul(yt[:], wo01_sb[:, ds], on01[:, qs],
                                             start=True, stop=False)
                            nc.tensor.matmul(yt[:], wo2_sb[:, ds], on2[:, qs],
                                             start=False, stop=True)
                            ysb = ysp.tile([128, 512], f32, tag=f"ysb{dc % 2}")
                            if dc % 2 == 0:
                                nc.vector.tensor_copy(ysb[:], yt[:])
                            else:
                                nc.scalar.copy(ysb[:], yt[:])
                            nc.sync.dma_start(y[ds, qs], ysb[:])



def _build(iters=1):
    import concourse.bacc as bacc
    import concourse.tile as tile
    from concourse import mybir

    f32 = mybir.dt.float32
    bf16 = mybir.dt.bfloat16
    nc = bacc.Bacc("TRN2", target_bir_lowering=False, debug=False, name="mhsa")

    tens = (
        nc.dram_tensor("xT", [D, T], bf16, kind="ExternalInput"),
        nc.dram_tensor("wqT", [D, HPC * DH], bf16, kind="ExternalInput"),
        nc.dram_tensor("wkT", [D, HPC * DH], bf16, kind="ExternalInput"),
        nc.dram_tensor("wvT", [D, HPC * DH], bf16, kind="ExternalInput"),
        nc.dram_tensor("bq", [HPC * DH, 1], f32, kind="ExternalInput"),
        nc.dram_tensor("bk", [HPC * DH, 1], f32, kind="ExternalInput"),
        nc.dram_tensor("bv", [1, HPC * DH], bf16, kind="ExternalInput"),
        nc.dram_tensor("wo01", [128, D], bf16, kind="ExternalInput"),
        nc.dram_tensor("wo2", [DH + 1, D], bf16, kind="ExternalInput"),
        nc.dram_tensor("onesb", [1, T], bf16, kind="ExternalInput"),
        nc.dram_tensor("ones65", [1, DH + 1], mybir.dt.float32r, kind="ExternalInput"),
        nc.dram_tensor("y", [D, T], f32, kind="ExternalOutput"),
    )
    with tile.TileContext(nc) as tc:
        _trace(nc, tc, mybir, tens, iters)
    nc.finalize()
    return nc


def _prep_inputs(x, Wq, bq, Wk, bk, Wv, bv, Wo, bo):
    in_maps = []
    xTb = [np.ascontiguousarray(x[b].T).astype(BF16) for b in range(B)]
    for c in range(NCORES):
        b = c // 4
        h0 = (c % 4) * HPC
        cols = slice(h0 * DH, (h0 + HPC) * DH)
        woT = np.ascontiguousarray(Wo[:, cols].T)  # [192, 768]
        wo01 = np.ascontiguousarray(woT[0:128]).astype(BF16)
        wo2 = np.zeros((DH + 1, D), dtype=BF16)
        wo2[0:DH] = woT[2 * DH : 3 * DH].astype(BF16)
        if c % 4 == 0:
            wo2[DH] = bo.astype(BF16)
        in_maps.append(
            {
                "xT": xTb[b],
                "wqT": np.ascontiguousarray(Wq[cols, :].T * EXP_ALPHA).astype(BF16),
                "wkT": np.ascontiguousarray(Wk[cols, :].T).astype(BF16),
                "wvT": np.ascontiguousarray(Wv[cols, :].T).astype(BF16),
                "bq": np.ascontiguousarray(bq[cols] * EXP_ALPHA)
                .reshape(-1, 1).astype(np.float32),
                "bk": np.ascontiguousarray(bk[cols]).reshape(-1, 1).astype(np.float32),
                "bv": np.ascontiguousarray(bv[cols]).reshape(1, -1).astype(BF16),
                "wo01": wo01,
                "wo2": wo2,
                "onesb": np.ones((1, T), dtype=BF16),
                "ones65": np.ones((1, DH + 1), dtype=np.float32),
            }
        )
    return in_maps


def kernel(x, Wq, bq, Wk, bk, Wv, bv, Wo, bo):
    x = np.asarray(x, dtype=np.float32)
    Wq, bq = np.asarray(Wq, np.float32), np.asarray(bq, np.float32)
    Wk, bk = np.asarray(Wk, np.float32), np.asarray(bk, np.float32)
    Wv, bv = np.asarray(Wv, np.float32), np.asarray(bv, np.float32)
    Wo, bo = np.asarray(Wo, np.float32), np.asarray(bo, np.float32)

    from concourse.bass_utils import run_bass_kernel_spmd

    iters = int(os.environ.get("MHSA_ITERS", "1"))
    key = ("nc", iters)
    if key not in _CACHE:
        _CACHE[key] = _build(iters)
    nc = _CACHE[key]

    in_maps = _prep_inputs(x, Wq, bq, Wk, bk, Wv, bv, Wo, bo)
    trace = bool(os.environ.get("MHSA_TRACE"))
    res = run_bass_kernel_spmd(
        nc, in_maps, core_ids=list(range(NCORES)), trace=trace
    )
    if res.exec_time_ns is not None:
        print(f"HW exec time: {res.exec_time_ns} ns")
        _CACHE["exec_time_ns"] = res.exec_time_ns
        _CACHE["trace"] = res.instructions_and_trace

    out = np.zeros((B, T, D), dtype=np.float32)
    for c in range(NCORES):
        out[c // 4] += res.results[c]["y"].T
    return out



# revision 29
# speedup vs baseline: 1.7205x; 1.7205x over previous
"""Multi-head self-attention (B=2, T=4096, D=768, H=12) on 8 TRN2 NeuronCores.

Sharding: (batch, head)-parallel. Core c (0..7) handles batch b=c//4 and the
3 heads h0=(c%4)*3 .. h0+2.  Each core computes Q/K/V projections for its
heads, full softmax(QK^T/sqrt(d))V attention, and a partial output projection
through its 192 rows of Wo.  The host sums the 4 partials per batch
(row-sharded Wo all-reduce done host-side); bo rides an augmented ones-row
on core c%4==0 only.

Per-core device pipeline:
  x.T [768,4096] -> Qt/Kt in [d,t] layout (heads pair-packed into 128
  partitions), V in natural [t,d] layout augmented with a leading ones
  column.  Scores are computed transposed, S^T[k,q], with two concurrent
  row-tiled matmuls (K=64 each); ACT applies exp(scale*s) PSUM->SBUF bf16;
  PV accumulates O'[65,512] = sum_k V'[k]^T P[k] where row 0 is the softmax
  denominator; a K=1 broadcast matmul + approx reciprocal + multiply
  normalizes; the Wo projection consumes the normalized O^T tiles directly.
"""

import os
import numpy as np
import ml_dtypes

B, T, D = 2, 4096, 768
H, DH = 12, 64
NCORES = 8
HPC = 3            # heads per core
KC = D // 128      # 6 contraction chunks for projections
NT = T // 512      # 8 q/n tiles of 512
TT = T // 128      # 32 k/t tiles of 128
EXPG = 2           # k-tiles per exp instruction (PSUM banks per S tile)

BF16 = ml_dtypes.bfloat16

_CACHE = {}

# Custom-DVE exp approximation on pre-scaled scores v = ALPHA * u:
#   out = (((v + C0)*v + C1)*v + C2)^4  ~=  K4 * exp(u/8),  |u/8| <= ~2.6
# The monic cubic comes from folding the leading minimax coefficient into
# the Q projection weights (ALPHA), so the op needs only 3 scalars and a
# single tensor stream.  The tiny global factor K4 cancels in softmax;
# the ACT path matches it via its free bias so both engines agree.
EXP_ALPHA = 0.017076609845819633
EXPC0 = 1.7316677926445943
EXPC1 = 1.8336644492020884
EXPC2 = 0.9992203744053143
ACT_SCALE = (64 ** -0.5) / EXP_ALPHA  # exp(u/8) with u = v/ALPHA
_EXP_SHA_V3 = "4700acb2322239e1"


def _register_exp_op():
    from concourse import dve_ops
    from concourse.dve_spec import C0, C1, C2, Spec, Src0, sq

    for o in dve_ops.OPS:
        if o.name == "EXP_MONIC4_ANT":
            return o

    p = ((Src0 + C0) * Src0 + C1) * Src0 + C2
    body = sq(sq(p))

    def ref(in0, in1, s0, s1, imm2):
        x = in0.astype(np.float32)
        pv = ((x + s0) * x + s1) * x + imm2
        return ((pv * pv) ** 2).astype(np.float32)

    op = dve_ops.DveOp(
        "EXP_MONIC4_ANT",
        Spec(body=body, reference=ref),
        subdim=False,
        uops_sha={"v3": _EXP_SHA_V3},
    )
    dve_ops.OPS.append(op)
    dve_ops._SUB_OPCODE_FOR_NAME[op.name] = (
        dve_ops._CUSTOM_DVE_ROW_BASE + len(dve_ops.OPS) - 1
    )
    dve_ops.CUSTOM_DVE_SPECS[op.name] = op.spec
    return op


def _trace(nc, tc, mybir, tens, iters=1):
    skip = os.environ.get("MHSA_SKIP", "")
    import concourse.bass as bass
    from contextlib import ExitStack

    f32 = mybir.dt.float32
    bf16 = mybir.dt.bfloat16
    Exp = mybir.ActivationFunctionType.Exp
    PSUM = bass.MemorySpace.PSUM
    NG = TT // EXPG  # exp groups per (head, qtile) in the paired path
    exp_op = _register_exp_op()

    with ExitStack() as ctx:
        persist = ctx.enter_context(tc.tile_pool(name="persist", bufs=1))

        # ---- persistent SBUF ----
        x_ch = [
            persist.tile([128, T], bf16, name=f"xc{kc}") for kc in range(KC)
        ]
        w_q = persist.tile([128, KC, HPC * DH], bf16)
        w_k = persist.tile([128, KC, HPC * DH], bf16)
        w_v = persist.tile([128, KC, HPC * DH], bf16)
        bq01 = persist.tile([128, 1], f32)
        bq2 = persist.tile([64, 1], f32)
        bk01 = persist.tile([128, 1], f32)
        bk2v = persist.tile([128, 1], f32)  # h2 K bias on partitions 64:128
        bv_sb = persist.tile([1, HPC * DH], bf16)
        ones1 = persist.tile([1, 128], bf16)     # K=1 lhsT for V bias MM
        f32r = mybir.dt.float32r
        ones65 = persist.tile([DH + 1, DH + 1], f32r)  # row 64: K=1 denom bcast lhsT
        q01 = persist.tile([128, T], bf16)       # h0 rows 0:64, h1 rows 64:128
        k01 = persist.tile([128, T], bf16)
        q2 = persist.tile([128, T], bf16)        # h2, duplicated to rows 64:128
        k2 = persist.tile([128, T], bf16)
        v_sb = persist.tile([128, TT, HPC, 68], bf16)  # [1|V] per head
        # normalized O^T: on01 rows 0:64 = h0, 64:128 = h1; on2 rows 0:64 = h2,
        # row 64 = ones (rides bo through the phase-3 K=65 matmul)
        on01 = persist.tile([128, T], bf16)
        on2 = persist.tile([DH + 1, T], bf16)
        wo01_sb = persist.tile([128, D], bf16)
        wo2_sb = persist.tile([DH + 1, D], bf16)

        nc.vector.memset(v_sb[:, :, :, 64:65], 1.0)

        dve_share = float(os.environ.get("MHSA_DVE_SHARE", "0.5"))
        exp_acc = [0.0]

        def do_exp(dst, src):
            exp_acc[0] += dve_share
            if exp_acc[0] >= 1.0 - 1e-9:
                exp_acc[0] -= 1.0
                nc.vector._custom_dve(
                    exp_op, out=dst, in0=src,
                    s0=EXPC0, s1=EXPC1, imm2=EXPC2,
                )
            else:
                nc.scalar.activation(dst, src, Exp, scale=ACT_SCALE)

        # ---- input DMAs ----
        xT, wqT, wkT, wvT, bq, bk, bv, wo01, wo2, onesb, ones65d, y = tens
        nc.sync.dma_start(ones1[:], onesb[0:1, 0:128])
        nc.sync.dma_start(on2[DH : DH + 1, :], onesb[:])
        nc.sync.dma_start(ones65[DH : DH + 1, :], ones65d[:])
        for kc in range(KC):
            r = slice(kc * 128, (kc + 1) * 128)
            nc.sync.dma_start(x_ch[kc][:], xT[r, :])
            nc.sync.dma_start(w_q[:, kc, :], wqT[r, :])
            nc.sync.dma_start(w_k[:, kc, :], wkT[r, :])
            nc.sync.dma_start(w_v[:, kc, :], wvT[r, :])
        nc.sync.dma_start(bq01[:], bq[0:128, :])
        nc.sync.dma_start(bq2[:], bq[128:192, :])
        nc.sync.dma_start(bk01[:], bk[0:128, :])
        nc.sync.dma_start(bk2v[64:128, :], bk[128:192, :])
        nc.sync.dma_start(bv_sb[:], bv[:])
        nc.sync.dma_start(wo01_sb[:], wo01[:])
        nc.sync.dma_start(wo2_sb[:], wo2[:])

        loop_cm = tc.For_i(0, iters, 1) if iters > 1 else None
        from contextlib import nullcontext
        with (loop_cm if loop_cm is not None else nullcontext()):
            # ---- Phase 1a: Q/K projections into [d, t] layout ----
            # pqb/pkb (64 rows each) run col-tiled concurrently in one pass
            with tc.tile_pool(name="pj", bufs=2, space=PSUM) as pj:
                for nt in range(NT):
                    s = slice(nt * 512, (nt + 1) * 512)
                    pqa = pj.tile([128, 512], f32, tag="pqa")
                    pka = pj.tile([128, 512], f32, tag="pka")
                    # h2 halves in separate banks but distinct col groups so
                    # the two M=64 matmul streams run concurrently on the PE
                    pq2 = pj.tile([128, 512], f32, tag="pq2")
                    pk2 = pj.tile([128, 512], f32, tag="pk2")
                    for kc in range(KC):
                        st, sp = kc == 0, kc == KC - 1
                        rhs = x_ch[kc][:, s]
                        nc.tensor.matmul(pqa[:], w_q[:, kc, 0:128], rhs, start=st, stop=sp)
                        nc.tensor.matmul(pka[:], w_k[:, kc, 0:128], rhs, start=st, stop=sp)
                        nc.tensor.matmul(pq2[0:64, :], w_q[:, kc, 128:192], rhs,
                                         start=st, stop=sp, tile_position=(0, 0))
                        nc.tensor.matmul(pk2[64:128, :], w_k[:, kc, 128:192], rhs,
                                         start=st, stop=sp, tile_position=(0, 64))
                    nc.vector.tensor_scalar_add(q01[:, s], pqa[:], bq01[:])
                    nc.vector.tensor_scalar_add(k01[:, s], pka[:], bk01[:])
                    nc.vector.tensor_scalar_add(q2[0:64, s], pq2[0:64, :], bq2[:])
                    nc.vector.tensor_scalar_add(k2[64:128, s], pk2[64:128, :],
                                                bk2v[64:128, :])

            # ---- Phase 1b: V projection into natural [t, d] layout ----
            with tc.tile_pool(name="pv", bufs=4, space=PSUM) as pvp:
                for tt in range(TT):
                    ts_ = slice(tt * 128, (tt + 1) * 128)
                    pvt = pvp.tile([128, HPC * DH], f32, tag="pvt")
                    nc.tensor.matmul(pvt[:], ones1[:], bv_sb[:], start=True, stop=False)
                    for kc in range(KC):
                        nc.tensor.matmul(
                            pvt[:], x_ch[kc][:, ts_], w_v[:, kc, :],
                            start=False, stop=kc == KC - 1,
                        )
                    nc.vector.tensor_copy(
                        v_sb[:, tt, :, 0:64],
                        pvt[:].rearrange("p (h d) -> p h d", h=HPC),
                    )

            # duplicate h2's Q/K to the other partition half for self-pairing
            nc.sync.dma_start(q2[64:128, :], q2[0:64, :])
            nc.sync.dma_start(k2[0:64, :], k2[64:128, :])

            # ---- Phase 2: attention per q-tile (exp split ACT/DVE) ----
            # DVE does ONLY exp + the short normalize tail here; everything
            # heavy (output projection, PSUM evacuation) moves to phase 3 so
            # the strict-FIFO DVE queue never blocks the next q-tile's exps.
            with (
                tc.tile_pool(name="spool", bufs=1, space=PSUM) as spool,
                tc.tile_pool(name="opool", bufs=1, space=PSUM) as opool,
                tc.tile_pool(name="mpool", bufs=1, space=PSUM) as mpool,
                tc.tile_pool(name="ppool", bufs=3) as ppool,
                tc.tile_pool(name="npool", bufs=2) as npool,
            ):
                for qt in range(NT):
                    qs = slice(qt * 512, (qt + 1) * 512)

                    def s_group(g, emit_pair):
                        """Emit EXPG steps of (2 score MMs) -> 2 exps -> caller PV."""
                        st = [
                            spool.tile([128, EXPG * 512], f32, tag=f"s{i}", name=f"s{i}")
                            for i in (0, 1)
                        ]
                        for j in range(EXPG):
                            emit_pair(st, g * EXPG + j, j)
                        pt = [
                            ppool.tile([128, EXPG * 512], bf16, tag=f"p{i}", name=f"p{i}")
                            for i in (0, 1)
                        ]
                        if "exp" not in skip:
                            do_exp(pt[0][:], st[0][:])
                            do_exp(pt[1][:], st[1][:])
                        return pt

                    # --- h0/h1 concurrently (row strips 0:64 / 64:128) ---
                    ol = [opool.tile([DH + 1, 512], f32, tag=f"o{i}", name=f"ol{i}") for i in (0, 1)]

                    def pair01(st, kt, j):
                        ks = slice(kt * 128, (kt + 1) * 128)
                        js = slice(j * 512, (j + 1) * 512)
                        nc.tensor.matmul(st[0][:, js], k01[0:64, ks], q01[0:64, qs],
                                         start=True, stop=True)
                        nc.tensor.matmul(st[1][:, js], k01[64:128, ks], q01[64:128, qs],
                                         start=True, stop=True)

                    for g in range(NG):
                        pt = s_group(g, pair01)
                        for j in range(EXPG if "pv" not in skip else 0):
                            kt = g * EXPG + j
                            js = slice(j * 512, (j + 1) * 512)
                            first = kt == 0
                            last = kt == TT - 1
                            nc.tensor.matmul(ol[0][:], v_sb[:, kt, 0, 0:65], pt[0][:, js],
                                             start=first, stop=last, skip_group_check=True)
                            nc.tensor.matmul(ol[1][:], v_sb[:, kt, 1, 0:65], pt[1][:, js],
                                             start=first, stop=last, skip_group_check=True)

                    norm_jobs = [(ol[0], on01[0:DH, :]), (ol[1], on01[DH : 2 * DH, :])]

                    # --- h2: even k-tiles on rows 0:64, odd on rows 64:128;
                    # both halves accumulate into ONE psum tile ---
                    o2 = opool.tile([DH + 1, 512], f32, tag="o2", name="o2")

                    def pair2(st, p, j):
                        ka = slice(2 * p * 128, (2 * p + 1) * 128)
                        kb = slice((2 * p + 1) * 128, (2 * p + 2) * 128)
                        js = slice(j * 512, (j + 1) * 512)
                        nc.tensor.matmul(st[0][:, js], k2[0:64, ka], q2[0:64, qs],
                                         start=True, stop=True)
                        nc.tensor.matmul(st[1][:, js], k2[64:128, kb], q2[64:128, qs],
                                         start=True, stop=True)

                    for g in range(NG // 2):
                        pt = s_group(g, pair2)
                        for j in range(EXPG if "pv" not in skip else 0):
                            p = g * EXPG + j
                            js = slice(j * 512, (j + 1) * 512)
                            nc.tensor.matmul(o2[:], v_sb[:, 2 * p, 2, 0:65], pt[0][:, js],
                                             start=p == 0, stop=False,
                                             skip_group_check=True)
                            nc.tensor.matmul(o2[:], v_sb[:, 2 * p + 1, 2, 0:65],
                                             pt[1][:, js],
                                             start=False, stop=p == TT // 2 - 1,
                                             skip_group_check=True)
                    if "pv" not in skip:
                        norm_jobs.append((o2, on2[0:DH, :]))
                    else:
                        norm_jobs = []

                    # --- normalize: broadcast denom row via K=1 matmul,
                    # reciprocal into SBUF, then mul (psum x sbuf) per head ---
                    for o_acc, on in norm_jobs:
                        lrow = npool.tile([DH + 1, 512], f32r, tag="lr")
                        nc.vector.tensor_copy(lrow[DH : DH + 1, :], o_acc[DH : DH + 1, :])
                        bc = mpool.tile([DH, 512], f32, tag="bc")
                        nc.tensor.matmul(
                            bc[:],
                            ones65[DH : DH + 1, 0:DH],
                            lrow[DH : DH + 1, :],
                            start=True, stop=True,
                        )
                        rc = npool.tile([DH, 512], f32, tag="rc")
                        nc.vector.reciprocal_approx_fast(rc[:], bc[:])
                        nc.vector.tensor_mul(on[:, qs], o_acc[0:DH, :], rc[:])

            # ---- Phase 3: output projection, y^T = Wo'^T @ O^T ----
            # K=128 (h0|h1) + K=65 (h2 + bias row) accumulating matmuls per
            # 128-wide d-chunk; PSUM evacuation split across ACT and DVE.
            if "wo" not in skip:
                with (
                    tc.tile_pool(name="ypsum", bufs=2, space=PSUM) as ypp,
                    tc.tile_pool(name="ysbp", bufs=3) as ysp,
                ):
                    for qt in range(NT):
                        qs = slice(qt * 512, (qt + 1) * 512)
                        for dc in range(D // 128):
                            ds = slice(dc * 128, (dc + 1) * 128)
                            yt = ypp.tile([128, 512], f32, tag="yt")
                            nc.tensor.matmul(yt[:], wo01_sb[:, ds], on01[:, qs],
                                             start=True, stop=False)
                            nc.tensor.matmul(yt[:], wo2_sb[:, ds], on2[:, qs],
                                             start=False, stop=True)
                            ysb = ysp.tile([128, 512], f32, tag=f"ysb{dc % 2}")
                            if dc % 2 == 0:
                                nc.vector.tensor_copy(ysb[:], yt[:])
                            else:
                                nc.scalar.copy(ysb[:], yt[:])
                            nc.sync.dma_start(y[ds, qs], ysb[:])



def _build(iters=1):
    import concourse.bacc as bacc
    import concourse.tile as tile
    from concourse import mybir

    f32 = mybir.dt.float32
    bf16 = mybir.dt.bfloat16
    nc = bacc.Bacc("TRN2", target_bir_lowering=False, debug=False, name="mhsa")

    tens = (
        nc.dram_tensor("xT", [D, T], bf16, kind="ExternalInput"),
        nc.dram_tensor("wqT", [D, HPC * DH], bf16, kind="ExternalInput"),
        nc.dram_tensor("wkT", [D, HPC * DH], bf16, kind="ExternalInput"),
        nc.dram_tensor("wvT", [D, HPC * DH], bf16, kind="ExternalInput"),
        nc.dram_tensor("bq", [HPC * DH, 1], f32, kind="ExternalInput"),
        nc.dram_tensor("bk", [HPC * DH, 1], f32, kind="ExternalInput"),
        nc.dram_tensor("bv", [1, HPC * DH], bf16, kind="ExternalInput"),
        nc.dram_tensor("wo01", [128, D], bf16, kind="ExternalInput"),
        nc.dram_tensor("wo2", [DH + 1, D], bf16, kind="ExternalInput"),
        nc.dram_tensor("onesb", [1, T], bf16, kind="ExternalInput"),
        nc.dram_tensor("ones65", [1, DH + 1], mybir.dt.float32r, kind="ExternalInput"),
        nc.dram_tensor("y", [D, T], f32, kind="ExternalOutput"),
    )
    with tile.TileContext(nc) as tc:
        _trace(nc, tc, mybir, tens, iters)
    nc.finalize()
    return nc


def _prep_inputs(x, Wq, bq, Wk, bk, Wv, bv, Wo, bo):
    in_maps = []
    xTb = [np.ascontiguousarray(x[b].T).astype(BF16) for b in range(B)]
    for c in range(NCORES):
        b = c // 4
        h0 = (c % 4) * HPC
        cols = slice(h0 * DH, (h0 + HPC) * DH)
        woT = np.ascontiguousarray(Wo[:, cols].T)  # [192, 768]
        wo01 = np.ascontiguousarray(woT[0:128]).astype(BF16)
        wo2 = np.zeros((DH + 1, D), dtype=BF16)
        wo2[0:DH] = woT[2 * DH : 3 * DH].astype(BF16)
        if c % 4 == 0:
            wo2[DH] = bo.astype(BF16)
        in_maps.append(
            {
                "xT": xTb[b],
                "wqT": np.ascontiguousarray(Wq[cols, :].T * EXP_ALPHA).astype(BF16),
                "wkT": np.ascontiguousarray(Wk[cols, :].T).astype(BF16),
                "wvT": np.ascontiguousarray(Wv[cols, :].T).astype(BF16),
                "bq": np.ascontiguousarray(bq[cols] * EXP_ALPHA)
                .reshape(-1, 1).astype(np.float32),
                "bk": np.ascontiguousarray(bk[cols]).reshape(-1, 1).astype(np.float32),
                "bv": np.ascontiguousarray(bv[cols]).reshape(1, -1).astype(BF16),
                "wo01": wo01,
                "wo2": wo2,
                "onesb": np.ones((1, T), dtype=BF16),
                "ones65": np.ones((1, DH + 1), dtype=np.float32),
            }
        )
    return in_maps


def kernel(x, Wq, bq, Wk, bk, Wv, bv, Wo, bo):
    x = np.asarray(x, dtype=np.float32)
    Wq, bq = np.asarray(Wq, np.float32), np.asarray(bq, np.float32)
    Wk, bk = np.asarray(Wk, np.float32), np.asarray(bk, np.float32)
    Wv, bv = np.asarray(Wv, np.float32), np.asarray(bv, np.float32)
    Wo, bo = np.asarray(Wo, np.float32), np.asarray(bo, np.float32)

    from concourse.bass_utils import run_bass_kernel_spmd

    iters = int(os.environ.get("MHSA_ITERS", "1"))
    key = ("nc", iters)
    if key not in _CACHE:
        _CACHE[key] = _build(iters)
    nc = _CACHE[key]

    in_maps = _prep_inputs(x, Wq, bq, Wk, bk, Wv, bv, Wo, bo)
    trace = bool(os.environ.get("MHSA_TRACE"))
    res = run_bass_kernel_spmd(
        nc, in_maps, core_ids=list(range(NCORES)), trace=trace
    )
    if res.exec_time_ns is not None:
        print(f"HW exec time: {res.exec_time_ns} ns")
        _CACHE["exec_time_ns"] = res.exec_time_ns
        _CACHE["trace"] = res.instructions_and_trace

    out = np.zeros((B, T, D), dtype=np.float32)
    for c in range(NCORES):
        out[c // 4] += res.results[c]["y"].T
    return out

